# revision 3
# baseline (speedup 1.0000x reference)
"""AgentAttention Trainium2 kernel: 8-core data-parallel over batch.

v2: cross-batch software pipeline. The TRN2 PE runs at 1.2GHz until it has
been continuously busy for ~3us, then 2.4GHz. The v1 kernel ran each batch's
attention stages serially, so the PE idled at every tensor<->scalar/vector
handoff and spent the whole attention phase at half clock. Here the dense
qkv GEMM units of batch b+1 are interleaved as fillers between the
dependency hops of batch b's attention, keeping the PE stream gap-free.

Layouts (per core, 4 batches):
  xT      [4, 768, 1176] bf16  (c-major x)
  qkT     c-major q,k: 12 sbuf tiles [128, 1176] (tiles 0-5 = q, 6-11 = k)
  v_ext   pos-major v with per-head ones column (col 64): 10 tiles [128, 12*65]
  agT     pooled agent tokens (sums over 4x4 blocks), c-major [128, 49] x6
  aoT     c-major attention output (bf16) [128, 1176] x6 -> proj -> out
Matmuls bf16, fp32 psum (uniform pool of 8 one-bank tiles [128,512]).
Softmax scale folded into ACT exp scale (0.125 stage1; 0.125/16 stages 2/3
-- agent tokens are pooled SUMS). qk bias via per-partition tensor_scalar.
"""

import sys

sys.path.insert(0, "/opt/trn_rl_repo")

import numpy as np
import ml_dtypes

import concourse.bass as bass
import concourse.mybir as mybir
import concourse.tile as tile
from concourse import bacc, bass_utils
from concourse.masks import make_identity

BF = mybir.dt.bfloat16
F32 = mybir.dt.float32
AF = mybir.ActivationFunctionType

N_CORES = 8
B, N, C = 32, 1176, 768
NB = B // N_CORES
H, HD = 12, 64
N_MT, N_S = 392, 784
A = 49
SCALE1 = 0.125
SCALE23 = 0.125 / 16.0

POS_T = [(pt * 128, min(128, N - pt * 128)) for pt in range(10)]
KEY1_T = [(0, 128), (128, 128), (256, 128), (384, 8)]
NCHUNK = [(0, 392), (392, 392), (784, 392)]
CCHUNK = [(0, 512), (512, 256)]
TSP = 116  # transpose chunk col spacing (>=113, even)


def build_program():
    nc = bacc.Bacc("TRN2", debug=False, num_devices=N_CORES)

    xT_d = nc.dram_tensor("xT", [NB, C, N], BF, kind="ExternalInput").ap()
    wqkT_d = nc.dram_tensor("wqkT", [C, 3 * C], BF, kind="ExternalInput").ap()
    wpjT_d = nc.dram_tensor("wpjT", [C, C], BF, kind="ExternalInput").ap()
    bqk_d = nc.dram_tensor("bqk", [1, 3 * C], BF, kind="ExternalInput").ap()
    bqkp_d = nc.dram_tensor("bqkp", [128, 12], F32, kind="ExternalInput").ap()
    bpj_d = nc.dram_tensor("bpj", [1, C], F32, kind="ExternalInput").ap()
    out_d = nc.dram_tensor("out", [NB, N, C], F32, kind="ExternalOutput").ap()

    with tc_ctx(nc) as (tc, cpool, wpool, hpool, ppool):
        # ---- one-time constants/weights ----
        wq = [
            cpool.tile([128, 3 * C], BF, tag=f"wq{i}", name=f"wq{i}") for i in range(6)
        ]
        wp = [cpool.tile([128, C], BF, tag=f"wp{i}", name=f"wp{i}") for i in range(6)]
        for i in range(6):
            nc.sync.dma_start(wq[i][:], wqkT_d[128 * i : 128 * (i + 1), :])
            nc.sync.dma_start(wp[i][:], wpjT_d[128 * i : 128 * (i + 1), :])
        sb_bqk = cpool.tile([1, 3 * C], BF, tag="bqk")
        nc.sync.dma_start(sb_bqk[:], bqk_d[:])
        bqkp = cpool.tile([128, 12], F32, tag="bqkp")
        nc.sync.dma_start(bqkp[:], bqkp_d[:])
        vb_bc = cpool.tile([128, C], BF, tag="vb_bc")
        nc.gpsimd.partition_broadcast(vb_bc[:], sb_bqk[0:1, 2 * C : 3 * C])
        bpjf = cpool.tile([1, C], F32, tag="bpjf", name="bpjf")
        nc.sync.dma_start(bpjf[:], bpj_d[:])
        pb_bc = cpool.tile([128, C], F32, tag="pb_bc")
        nc.gpsimd.partition_broadcast(pb_bc[:], bpjf[0:1, :])
        ident = cpool.tile([128, 128], BF, tag="ident")
        make_identity(nc, ident[:])

        # per-batch tile handles (rotated via tags, bufs=2)
        xT = {}
        qkT = {}
        v_ext = {}
        agT = {}
        aoT = {}

        def psum(name):
            return ppool.tile([128, 512], F32, tag="P", name=name, bufs=8)

        def load_x(b):
            xT[b] = [
                hpool.tile([128, N], BF, tag=f"xT{i}", name=f"xT{i}", bufs=2)
                for i in range(6)
            ]
            for i in range(6):
                nc.sync.dma_start(xT[b][i][:], xT_d[b, 128 * i : 128 * (i + 1), :])

        def q_unit(b, m):
            # qkT[m] c-major [128, 1176] for q (m<6) / k (m>=6) rows
            if m == 0:
                qkT[b] = [None] * 12
            t = hpool.tile([128, N], BF, tag=f"qkT{m}", name=f"qkT{m}", bufs=2)
            qkT[b][m] = t
            for n0, nsz in NCHUNK:
                ps = psum("psQ")
                for kt in range(6):
                    nc.tensor.matmul(
                        ps[:, 0:nsz],
                        wq[kt][:, 128 * m : 128 * (m + 1)],
                        xT[b][kt][:, n0 : n0 + nsz],
                        start=(kt == 0),
                        stop=(kt == 5),
                    )
                nc.vector.tensor_scalar_add(
                    t[:, n0 : n0 + nsz], ps[:, 0:nsz], bqkp[:, m : m + 1]
                )

        def v_unit(b, pt):
            # pos-major v_ext [psz, 12*65] with ones col at 64 of each head
            p0, psz = POS_T[pt]
            if pt == 0:
                v_ext[b] = [None] * 10
            vt = hpool.tile([128, H * 65], BF, tag=f"vx{pt}", name=f"vx{pt}", bufs=2)
            v_ext[b][pt] = vt
            if b < 2:
                # two rotation slots; evac only writes the 64 v columns, so
                # ones persist across later batches
                nc.vector.memset(
                    vt[:].rearrange("p (h e) -> p h e", e=65)[:, :, 64:65], 1.0
                )
            for ci, (c0, csz) in enumerate(CCHUNK):
                ps = psum("psV")
                for kt in range(6):
                    nc.tensor.matmul(
                        ps[0:psz, 0:csz],
                        xT[b][kt][:, p0 : p0 + psz],
                        wq[kt][:, 2 * C + c0 : 2 * C + c0 + csz],
                        start=(kt == 0),
                        stop=(kt == 5),
                    )
                nh = csz // 64
                h0 = c0 // 64
                nc.vector.tensor_add(
                    vt[0:psz].rearrange("p (h e) -> p h e", e=65)[
                        :, h0 : h0 + nh, 0:64
                    ],
                    ps[0:psz, 0:csz].rearrange("p (h d) -> p h d", d=64),
                    vb_bc[0:psz, c0 : c0 + csz].rearrange("p (h d) -> p h d", d=64),
                )

        def pool_agents(b):
            # sum 4x4 blocks of q_s -> agT (c-major), on gpsimd
            agT[b] = []
            for ct in range(6):
                t1 = wpool.tile([128, 196], F32, tag="t1")
                qs = qkT[b][ct][:, N_MT:N]  # [128, 784], idx = i*28 + aj*4 + dj
                q4 = qs.rearrange("p (x dj) -> p x dj", dj=4)
                nc.gpsimd.tensor_add(t1[:, 0:196], q4[:, :, 0:1], q4[:, :, 1:2])
                nc.gpsimd.tensor_add(t1[:, 0:196], t1[:, 0:196], q4[:, :, 2:3])
                nc.gpsimd.tensor_add(t1[:, 0:196], t1[:, 0:196], q4[:, :, 3:4])
                ag = hpool.tile([128, A], BF, tag=f"ag{ct}", name=f"ag{ct}", bufs=2)
                agT[b].append(ag)
                # t1 idx = 28*ai + 7*di + aj -> view (ai, aj, di)
                t4 = t1[:, 0:196].rearrange("p (ai di aj) -> p ai aj di", ai=7, di=4)
                t2 = wpool.tile([128, A], F32, tag="t2")
                nc.gpsimd.tensor_add(t2[:, 0:A], t4[:, :, :, 0:1], t4[:, :, :, 1:2])
                nc.gpsimd.tensor_add(t2[:, 0:A], t2[:, 0:A], t4[:, :, :, 2:3])
                nc.gpsimd.tensor_add(ag[:, 0:A], t2[:, 0:A], t4[:, :, :, 3:4])

        def norm_chain(pv, dst):
            # dst = pv[0:64] / pv[64] (per free-dim query), pv is psum
            rc = wpool.tile([1, 392], F32, tag="rc", bufs=2)
            nc.vector.reciprocal_approx_fast(out=rc[:, 0:392], in_=pv[64:65, 0:392])
            bc = wpool.tile([64, 392], F32, tag="bc", bufs=2)
            nc.gpsimd.partition_broadcast(bc[:], rc[0:1, 0:392])
            nc.vector.tensor_mul(dst, pv[0:64, 0:392], bc[:])

        # ---- attention for one head pair, split into schedulable chunks ----
        def pair_scores(b, p2, st):
            qt = p2
            # stage 2 scores: [49x2 packed, keys] over 3 chunks
            st["s2"] = []
            for n0, nsz in NCHUNK:
                ps = psum("psS2")
                st["s2"].append(ps)
                for hp in range(2):
                    qo = 64 * hp
                    nc.tensor.matmul(
                        ps[qo : qo + 49, 0:nsz],
                        agT[b][qt][qo : qo + 64, 0:A],
                        qkT[b][6 + qt][qo : qo + 64, n0 : n0 + nsz],
                        start=True,
                        stop=True,
                    )
            # stage 3 scores: [49x2 packed (agents), queries] over 2 chunks
            st["s3"] = []
            for cc in range(2):
                ps = psum("psS3")
                st["s3"].append(ps)
                for hp in range(2):
                    qo = 64 * hp
                    nc.tensor.matmul(
                        ps[qo : qo + 49, 0:392],
                        agT[b][qt][qo : qo + 64, 0:A],
                        qkT[b][qt][qo : qo + 64, N_MT + 392 * cc : N_MT + 392 * (cc + 1)],
                        start=True,
                        stop=True,
                    )
            # stage 1 scores: [keys, queries] per head over 4 key chunks
            st["s1"] = []
            for hp in range(2):
                qo = 64 * hp
                chunks = []
                st["s1"].append(chunks)
                for k0, ksz in KEY1_T:
                    ps = psum("psS1")
                    chunks.append(ps)
                    nc.tensor.matmul(
                        ps[0:ksz, 0:392],
                        qkT[b][6 + qt][qo : qo + 64, k0 : k0 + ksz],
                        qkT[b][qt][qo : qo + 64, 0:N_MT],
                        start=True,
                        stop=True,
                    )
            # exps (scalar engine, in dependency-use order: e2, e3, e1)
            e2 = wpool.tile([128, N], BF, tag="e2")
            st["e2"] = e2
            for j, (n0, nsz) in enumerate(NCHUNK):
                nc.scalar.activation(
                    e2[0:113, n0 : n0 + nsz],
                    st["s2"][j][0:113, 0:nsz],
                    AF.Exp,
                    scale=SCALE23,
                )
            st["e3"] = []
            for cc in range(2):
                e3 = wpool.tile([128, 392], BF, tag="e3", name="e3", bufs=2)
                st["e3"].append(e3)
                nc.scalar.activation(
                    e3[0:113, 0:392], st["s3"][cc][0:113, 0:392], AF.Exp, scale=SCALE23
                )
            st["e1"] = []
            for hp in range(2):
                e1s = []
                st["e1"].append(e1s)
                for j, (k0, ksz) in enumerate(KEY1_T):
                    e1 = wpool.tile([128, 392], BF, tag="e1", name="e1", bufs=8)
                    e1s.append(e1)
                    nc.scalar.activation(
                        e1[0:ksz, 0:392],
                        st["s1"][hp][j][0:ksz, 0:392],
                        AF.Exp,
                        scale=SCALE1,
                    )

        def pair_pv1(b, p2, st):
            qt = p2
            for hp in range(2):
                qo = 64 * hp
                pv = psum("psPV1")
                for j, (k0, ksz) in enumerate(KEY1_T):
                    nc.tensor.matmul(
                        pv[0:65, 0:392],
                        v_ext[b][j][0:ksz, 65 * (2 * p2 + hp) : 65 * (2 * p2 + hp) + 65],
                        st["e1"][hp][j][0:ksz, 0:392],
                        start=(j == 0),
                        stop=(j == 3),
                    )
                norm_chain(pv, aoT[b][qt][qo : qo + 64, 0:N_MT])

        def pair_transp(b, p2, st):
            # [113, keys] -> [keys, 113] in 10 chunks, via identity matmul
            st["eT"] = []
            for half in range(2):
                trp = ppool.tile([128, 5 * TSP], BF, tag="P", name="psTr", bufs=8)
                for kk in range(5):
                    kt = 5 * half + kk
                    p0, psz = POS_T[kt]
                    nc.tensor.transpose(
                        trp[0:psz, TSP * kk : TSP * kk + 113],
                        st["e2"][0:113, p0 : p0 + psz],
                        ident[0:113, 0:113],
                    )
                eT = wpool.tile([128, 5 * TSP], BF, tag="e2T", bufs=2)
                st["eT"].append(eT)
                nc.vector.tensor_copy(eT[:, 0 : 5 * TSP], trp[:, 0 : 5 * TSP])

        def pair_pv2(b, p2, st):
            pv2 = psum("psPV2")
            for hp in range(2):
                h = 2 * p2 + hp
                o = 64 * hp
                for kt, (p0, psz) in enumerate(POS_T):
                    eT = st["eT"][kt // 5]
                    cof = TSP * (kt % 5) + 64 * hp
                    nc.tensor.matmul(
                        pv2[o : o + 49, 0:65],
                        eT[0:psz, cof : cof + 49],
                        v_ext[b][kt][0:psz, 65 * h : 65 * h + 65],
                        start=(kt == 0),
                        stop=(kt == 9),
                    )
            av = wpool.tile([128, 65], BF, tag="avx", bufs=2)
            st["av"] = av
            nc.vector.memset(av[0:113, 64:65], 1.0)
            avr = wpool.tile([128, 1], F32, tag="avr", bufs=2)
            nc.vector.reciprocal(avr[0:113, 0:1], pv2[0:113, 64:65])
            nc.vector.tensor_scalar_mul(
                av[0:113, 0:64], pv2[0:113, 0:64], avr[0:113, 0:1]
            )

        def pair_pv3(b, p2, st):
            qt = p2
            for hp in range(2):
                qo = 64 * hp
                for cc in range(2):
                    pv = psum("psPV3")
                    nc.tensor.matmul(
                        pv[0:65, 0:392],
                        st["av"][64 * hp : 64 * hp + 49, 0:65],
                        st["e3"][cc][64 * hp : 64 * hp + 49, 0:392],
                        start=True,
                        stop=True,
                    )
                    norm_chain(
                        pv,
                        aoT[b][qt][qo : qo + 64, N_MT + 392 * cc : N_MT + 392 * (cc + 1)],
                    )

        def proj_unit(b, pt):
            p0, psz = POS_T[pt]
            ob = wpool.tile([128, C], F32, tag="osb")
            for c0, csz in CCHUNK:
                ps = psum("psPJ")
                for kt in range(6):
                    nc.tensor.matmul(
                        ps[0:psz, 0:csz],
                        aoT[b][kt][:, p0 : p0 + psz],
                        wp[kt][:, c0 : c0 + csz],
                        start=(kt == 0),
                        stop=(kt == 5),
                    )
                nc.vector.tensor_add(
                    ob[0:psz, c0 : c0 + csz], ps[0:psz, 0:csz], pb_bc[0:psz, c0 : c0 + csz]
                )
            nc.sync.dma_start(out_d[b, p0 : p0 + psz, :], ob[0:psz, :])

        def qkv_units(b):
            units = []
            for m in range(12):
                units.append(lambda m=m: q_unit(b, m))
                if m == 5:
                    units.append(lambda: pool_agents(b))
            for pt in range(10):
                units.append(lambda pt=pt: v_unit(b, pt))
            return units

        # ---- schedule ----
        load_x(0)
        load_x(1)
        for u in qkv_units(0):
            u()

        for b in range(NB):
            fill = list(qkv_units(b + 1)) if b + 1 < NB else []
            if b + 2 < NB:
                load_x(b + 2)
            fi = 0

            def take(n):
                nonlocal fi
                for _ in range(n):
                    if fi < len(fill):
                        fill[fi]()
                        fi += 1

            aoT[b] = [
                hpool.tile([128, N], BF, tag=f"ao{i}", name=f"ao{i}", bufs=1)
                for i in range(6)
            ]
            for p2 in range(6):
                st = {}
                pair_scores(b, p2, st)
                take(1)
                pair_pv1(b, p2, st)
                pair_transp(b, p2, st)
                take(1)
                pair_pv2(b, p2, st)
                take(1)
                pair_pv3(b, p2, st)
            take(len(fill))
            for pt in range(10):
                proj_unit(b, pt)

    nc.compile()
    return nc


def tc_ctx(nc):
    from contextlib import contextmanager

    @contextmanager
    def ctx():
        with tile.TileContext(nc) as tc, nc.allow_low_precision(reason="attn bf16"):
            with (
                tc.tile_pool(name="const", bufs=1) as cpool,
                tc.tile_pool(name="work", bufs=2) as wpool,
                tc.tile_pool(name="hold", bufs=1) as hpool,
                tc.tile_pool(name="psum", bufs=8, space="PSUM") as ppool,
            ):
                yield tc, cpool, wpool, hpool, ppool

    return ctx()


_PROGRAM = None


def _get_program():
    global _PROGRAM
    if _PROGRAM is None:
        _PROGRAM = build_program()
    return _PROGRAM


def _prep_inputs(x, qkv_w, qkv_b, proj_w, proj_b):
    bf = ml_dtypes.bfloat16
    x = np.asarray(x, dtype=np.float32)
    xT = np.ascontiguousarray(x.transpose(0, 2, 1)).astype(bf)  # [B, C, N]
    wqkT = np.ascontiguousarray(np.asarray(qkv_w, dtype=np.float32).T).astype(bf)
    wpjT = np.ascontiguousarray(np.asarray(proj_w, dtype=np.float32).T).astype(bf)
    bqk = np.asarray(qkv_b, dtype=np.float32).reshape(1, -1).astype(bf)
    bqkp = np.ascontiguousarray(
        np.asarray(qkv_b, dtype=np.float32)[: 2 * 768].reshape(12, 128).T
    ).astype(np.float32)
    bpj = np.asarray(proj_b, dtype=np.float32).reshape(1, -1)
    in_maps = []
    for c in range(N_CORES):
        in_maps.append(
            {
                "xT": np.ascontiguousarray(xT[c * NB : (c + 1) * NB]),
                "wqkT": wqkT,
                "wpjT": wpjT,
                "bqk": bqk,
                "bqkp": bqkp,
                "bpj": bpj,
            }
        )
    return in_maps


def kernel(x, qkv_w, qkv_b, proj_w, proj_b, t_h=14, t_w=14, s_h=28, s_w=28, **kw):
    nc = _get_program()
    in_maps = _prep_inputs(x, qkv_w, qkv_b, proj_w, proj_b)
    res = bass_utils.run_bass_kernel_spmd(nc, in_maps, core_ids=list(range(N_CORES)))
    out = np.concatenate([res.results[c]["out"] for c in range(N_CORES)], axis=0)
    return out.astype(np.float32)


if __name__ == "__main__":
    build_program()
    print("program built OK")


# revision 15
# speedup vs baseline: 1.5273x; 1.5273x over previous
"""AgentAttention Trainium2 kernel: 8-core data-parallel over batch.

v2: cross-batch software pipeline. The TRN2 PE runs at 1.2GHz until it has
been continuously busy for ~3us, then 2.4GHz. The v1 kernel ran each batch's
attention stages serially, so the PE idled at every tensor<->scalar/vector
handoff and spent the whole attention phase at half clock. Here the dense
qkv GEMM units of batch b+1 are interleaved as fillers between the
dependency hops of batch b's attention, keeping the PE stream gap-free.

Layouts (per core, 4 batches):
  xT      [4, 768, 1176] bf16  (c-major x)
  qkT     c-major q,k: 12 sbuf tiles [128, 1176] (tiles 0-5 = q, 6-11 = k)
  v_ext   pos-major v with per-head ones column (col 64): 10 tiles [128, 12*65]
  agT     pooled agent tokens (sums over 4x4 blocks), c-major [128, 49] x6
  aoT     c-major attention output (bf16) [128, 1176] x6 -> proj -> out
Matmuls bf16, fp32 psum (uniform pool of 8 one-bank tiles [128,512]).
Softmax scale folded into ACT exp scale (0.125 stage1; 0.125/16 stages 2/3
-- agent tokens are pooled SUMS). qk bias via per-partition tensor_scalar.
"""

import sys

sys.path.insert(0, "/opt/trn_rl_repo")

import numpy as np
import ml_dtypes

import concourse.bass as bass
import concourse.mybir as mybir
import concourse.tile as tile
from concourse import bacc, bass_utils
from concourse.masks import make_identity

BF = mybir.dt.bfloat16
F32 = mybir.dt.float32
AF = mybir.ActivationFunctionType

N_CORES = 8
B, N, C = 32, 1176, 768
NB = B // N_CORES
H, HD = 12, 64
N_MT, N_S = 392, 784
A = 49
SCALE1 = 0.125
SCALE23 = 0.125 / 16.0

POS_T = [(pt * 128, min(128, N - pt * 128)) for pt in range(10)]
KEY1_T = [(0, 128), (128, 128), (256, 128), (384, 8)]
NCHUNK = [(0, 392), (392, 392), (784, 392)]
CCHUNK = [(0, 512), (512, 256)]
TSP = 116  # transpose chunk col spacing (>=113, even)


def build_program():
    nc = bacc.Bacc("TRN2", debug=False, num_devices=N_CORES)

    xT_d = nc.dram_tensor("xT", [NB, C, N], BF, kind="ExternalInput").ap()
    wqkT_d = nc.dram_tensor("wqkT", [C, 3 * C], BF, kind="ExternalInput").ap()
    wpjT_d = nc.dram_tensor("wpjT", [C, C], BF, kind="ExternalInput").ap()
    bqk_d = nc.dram_tensor("bqk", [1, 3 * C], BF, kind="ExternalInput").ap()
    bqkp_d = nc.dram_tensor("bqkp", [128, 12], F32, kind="ExternalInput").ap()
    bpj_d = nc.dram_tensor("bpj", [1, C], BF, kind="ExternalInput").ap()
    out_d = nc.dram_tensor("out", [NB, N, C], F32, kind="ExternalOutput").ap()

    with tc_ctx(nc) as (tc, cpool, wpool, hpool, ppool):
        # ---- one-time constants/weights ----
        wq = [
            cpool.tile([128, 3 * C], BF, tag=f"wq{i}", name=f"wq{i}") for i in range(6)
        ]
        wp = [cpool.tile([128, C], BF, tag=f"wp{i}", name=f"wp{i}") for i in range(6)]
        for i in range(6):
            nc.sync.dma_start(wq[i][:], wqkT_d[128 * i : 128 * (i + 1), :])
            nc.sync.dma_start(wp[i][:], wpjT_d[128 * i : 128 * (i + 1), :])
        sb_bqk = cpool.tile([1, 3 * C], BF, tag="bqk")
        nc.sync.dma_start(sb_bqk[:], bqk_d[:])
        bqkp = cpool.tile([128, 12], F32, tag="bqkp")
        nc.sync.dma_start(bqkp[:], bqkp_d[:])
        vb_bc = cpool.tile([128, C], BF, tag="vb_bc")
        nc.gpsimd.partition_broadcast(vb_bc[:], sb_bqk[0:1, 2 * C : 3 * C])
        bpjf = cpool.tile([1, C], BF, tag="bpjf", name="bpjf")
        nc.sync.dma_start(bpjf[:], bpj_d[:])
        pb_bc = cpool.tile([128, C], BF, tag="pb_bc")
        nc.gpsimd.partition_broadcast(pb_bc[:], bpjf[0:1, :])
        ident = cpool.tile([128, 128], BF, tag="ident")
        make_identity(nc, ident[:])

        # per-batch tile handles (rotated via tags, bufs=2)
        xT = {}
        qkT = {}
        v_ext = {}
        agT = {}
        aoT = {}

        def psum(name):
            return ppool.tile([128, 512], F32, tag="P", name=name, bufs=8)

        def load_x(b):
            xT[b] = [
                hpool.tile([128, N], BF, tag=f"xT{i}", name=f"xT{i}", bufs=2)
                for i in range(6)
            ]
            eng = nc.scalar if b == 0 else nc.sync
            for i in range(6):
                eng.dma_start(xT[b][i][:], xT_d[b, 128 * i : 128 * (i + 1), :])

        def q_unit(b, m):
            # qkT[m] c-major [128, 1176] for q (m<6) / k (m>=6) rows
            if m == 0:
                qkT[b] = [None] * 12
            t = hpool.tile([128, N], BF, tag=f"qkT{m}", name=f"qkT{m}", bufs=2)
            qkT[b][m] = t
            for n0, nsz in NCHUNK:
                ps = psum("psQ")
                for kt in range(6):
                    nc.tensor.matmul(
                        ps[:, 0:nsz],
                        wq[kt][:, 128 * m : 128 * (m + 1)],
                        xT[b][kt][:, n0 : n0 + nsz],
                        start=(kt == 0),
                        stop=(kt == 5),
                    )
                nc.vector.tensor_scalar_add(
                    t[:, n0 : n0 + nsz], ps[:, 0:nsz], bqkp[:, m : m + 1]
                )

        def v_unit(b, pt):
            # pos-major v_ext [psz, 12*65] with ones col at 64 of each head
            p0, psz = POS_T[pt]
            if pt == 0:
                v_ext[b] = [None] * 10
            vt = hpool.tile([128, H * 65], BF, tag=f"vx{pt}", name=f"vx{pt}", bufs=2)
            v_ext[b][pt] = vt
            if b < 2:
                # two rotation slots; evac only writes the 64 v columns, so
                # ones persist across later batches
                nc.vector.memset(
                    vt[:].rearrange("p (h e) -> p h e", e=65)[:, :, 64:65], 1.0
                )
            for ci, (c0, csz) in enumerate(CCHUNK):
                ps = psum("psV")
                for kt in range(6):
                    nc.tensor.matmul(
                        ps[0:psz, 0:csz],
                        xT[b][kt][:, p0 : p0 + psz],
                        wq[kt][:, 2 * C + c0 : 2 * C + c0 + csz],
                        start=(kt == 0),
                        stop=(kt == 5),
                    )
                nh = csz // 64
                h0 = c0 // 64
                nc.vector.tensor_add(
                    vt[0:psz].rearrange("p (h e) -> p h e", e=65)[
                        :, h0 : h0 + nh, 0:64
                    ],
                    ps[0:psz, 0:csz].rearrange("p (h d) -> p h d", d=64),
                    vb_bc[0:psz, c0 : c0 + csz].rearrange("p (h d) -> p h d", d=64),
                )

        def pool_ct(b, ct):
            # sum 4x4 blocks of q_s -> agT (c-major). On VECTOR: gpsimd must
            # stay broadcast-only (lib swaps + in-order blocking starve the
            # norm-chain broadcasts otherwise)
            if ct == 0:
                agT[b] = []
            t1 = wpool.tile([128, 196], F32, tag="t1", bufs=1)
            qs = qkT[b][ct][:, N_MT:N]  # [128, 784], idx = i*28 + aj*4 + dj
            q4 = qs.rearrange("p (x dj) -> p x dj", dj=4)
            nc.vector.tensor_add(t1[:, 0:196], q4[:, :, 0:1], q4[:, :, 1:2])
            nc.vector.tensor_add(t1[:, 0:196], t1[:, 0:196], q4[:, :, 2:3])
            nc.vector.tensor_add(t1[:, 0:196], t1[:, 0:196], q4[:, :, 3:4])
            ag = hpool.tile([128, A], BF, tag=f"ag{ct}", name=f"ag{ct}", bufs=2)
            agT[b].append(ag)
            # t1 idx = 28*ai + 7*di + aj -> view (ai, aj, di)
            t4 = t1[:, 0:196].rearrange("p (ai di aj) -> p ai aj di", ai=7, di=4)
            t2 = wpool.tile([128, A], F32, tag="t2")
            nc.vector.tensor_add(t2[:, 0:A], t4[:, :, :, 0:1], t4[:, :, :, 1:2])
            nc.vector.tensor_add(t2[:, 0:A], t2[:, 0:A], t4[:, :, :, 2:3])
            nc.vector.tensor_add(ag[:, 0:A], t2[:, 0:A], t4[:, :, :, 3:4])

        def norm_chain(pv, dst):
            # dst = pv[0:64] / pv[64] (per free-dim query), pv is psum
            se = wpool.tile([1, 392], F32, tag="se", bufs=1)
            nc.vector.tensor_copy(se[:, 0:392], pv[64:65, 0:392])
            rc = wpool.tile([1, 392], F32, tag="rc", bufs=2)
            nc.vector.reciprocal_approx_fast(out=rc[:, 0:392], in_=se[:, 0:392])
            bc = wpool.tile([64, 392], F32, tag="bc", bufs=2)
            nc.gpsimd.partition_broadcast(bc[:], rc[0:1, 0:392])
            nc.vector.tensor_mul(dst, pv[0:64, 0:392], bc[:])

        # ---- attention for one head pair, split into schedulable chunks ----
        def pair_scores(b, p2, st):
            qt = p2
            # stage 2 scores: [49x2 packed, keys] over 3 chunks
            st["s2"] = []
            for n0, nsz in NCHUNK:
                ps = psum("psS2")
                st["s2"].append(ps)
                for hp in range(2):
                    qo = 64 * hp
                    nc.tensor.matmul(
                        ps[qo : qo + 49, 0:nsz],
                        agT[b][qt][qo : qo + 64, 0:A],
                        qkT[b][6 + qt][qo : qo + 64, n0 : n0 + nsz],
                        start=True,
                        stop=True,
                    )
            # stage 3 scores: [49x2 packed (agents), queries] over 2 chunks
            st["s3"] = []
            for cc in range(2):
                ps = psum("psS3")
                st["s3"].append(ps)
                for hp in range(2):
                    qo = 64 * hp
                    nc.tensor.matmul(
                        ps[qo : qo + 49, 0:392],
                        agT[b][qt][qo : qo + 64, 0:A],
                        qkT[b][qt][qo : qo + 64, N_MT + 392 * cc : N_MT + 392 * (cc + 1)],
                        start=True,
                        stop=True,
                    )
            # stage 1 scores: [keys, queries] per head over 4 key chunks
            st["s1"] = []
            for hp in range(2):
                qo = 64 * hp
                chunks = []
                st["s1"].append(chunks)
                for k0, ksz in KEY1_T:
                    ps = psum("psS1")
                    chunks.append(ps)
                    nc.tensor.matmul(
                        ps[0:ksz, 0:392],
                        qkT[b][6 + qt][qo : qo + 64, k0 : k0 + ksz],
                        qkT[b][qt][qo : qo + 64, 0:N_MT],
                        start=True,
                        stop=True,
                    )
            # exps (scalar engine, in dependency-use order: e2, e3, e1)
            e2 = wpool.tile([128, N], BF, tag="e2")
            st["e2"] = e2
            for j, (n0, nsz) in enumerate(NCHUNK):
                nc.scalar.activation(
                    e2[0:113, n0 : n0 + nsz],
                    st["s2"][j][0:113, 0:nsz],
                    AF.Exp,
                    scale=SCALE23,
                )
            st["e3"] = []
            for cc in range(2):
                e3 = wpool.tile([128, 392], BF, tag="e3", name="e3", bufs=2)
                st["e3"].append(e3)
                nc.scalar.activation(
                    e3[0:113, 0:392], st["s3"][cc][0:113, 0:392], AF.Exp, scale=SCALE23
                )
            st["e1"] = []
            for hp in range(2):
                e1s = []
                st["e1"].append(e1s)
                for j, (k0, ksz) in enumerate(KEY1_T):
                    e1 = wpool.tile([128, 392], BF, tag="e1", name="e1", bufs=8)
                    e1s.append(e1)
                    nc.scalar.activation(
                        e1[0:ksz, 0:392],
                        st["s1"][hp][j][0:ksz, 0:392],
                        AF.Exp,
                        scale=SCALE1,
                    )

        def pair_pv1(b, p2, st):
            qt = p2
            for hp in range(2):
                qo = 64 * hp
                pv = psum("psPV1")
                for j, (k0, ksz) in enumerate(KEY1_T):
                    nc.tensor.matmul(
                        pv[0:65, 0:392],
                        v_ext[b][j][0:ksz, 65 * (2 * p2 + hp) : 65 * (2 * p2 + hp) + 65],
                        st["e1"][hp][j][0:ksz, 0:392],
                        start=(j == 0),
                        stop=(j == 3),
                    )
                norm_chain(pv, aoT[b][qt][qo : qo + 64, 0:N_MT])

        def pair_transp(b, p2, st):
            # [113, keys] -> [keys, 113] in 10 chunks, via identity matmul
            st["eT"] = []
            for half in range(2):
                trp = ppool.tile([128, 5 * TSP], BF, tag="P", name="psTr", bufs=8)
                for kk in range(5):
                    kt = 5 * half + kk
                    p0, psz = POS_T[kt]
                    nc.tensor.transpose(
                        trp[0:psz, TSP * kk : TSP * kk + 113],
                        st["e2"][0:113, p0 : p0 + psz],
                        ident[0:113, 0:113],
                    )
                eT = wpool.tile([128, 5 * TSP], BF, tag="e2T", bufs=2)
                st["eT"].append(eT)
                nc.vector.tensor_copy(eT[:, 0 : 5 * TSP], trp[:, 0 : 5 * TSP])

        def pair_pv2(b, p2, st):
            pv2 = psum("psPV2")
            for hp in range(2):
                h = 2 * p2 + hp
                o = 64 * hp
                for kt, (p0, psz) in enumerate(POS_T):
                    eT = st["eT"][kt // 5]
                    cof = TSP * (kt % 5) + 64 * hp
                    nc.tensor.matmul(
                        pv2[o : o + 49, 0:65],
                        eT[0:psz, cof : cof + 49],
                        v_ext[b][kt][0:psz, 65 * h : 65 * h + 65],
                        start=(kt == 0),
                        stop=(kt == 9),
                    )
            av = wpool.tile([128, 65], BF, tag="avx", bufs=2)
            st["av"] = av
            nc.vector.memset(av[0:113, 64:65], 1.0)
            avr = wpool.tile([128, 1], F32, tag="avr", bufs=2)
            nc.vector.reciprocal(avr[0:113, 0:1], pv2[0:113, 64:65])
            nc.vector.tensor_scalar_mul(
                av[0:113, 0:64], pv2[0:113, 0:64], avr[0:113, 0:1]
            )

        def pair_pv3(b, p2, st):
            qt = p2
            for hp in range(2):
                qo = 64 * hp
                for cc in range(2):
                    pv = psum("psPV3")
                    nc.tensor.matmul(
                        pv[0:65, 0:392],
                        st["av"][64 * hp : 64 * hp + 49, 0:65],
                        st["e3"][cc][64 * hp : 64 * hp + 49, 0:392],
                        start=True,
                        stop=True,
                    )
                    norm_chain(
                        pv,
                        aoT[b][qt][qo : qo + 64, N_MT + 392 * cc : N_MT + 392 * (cc + 1)],
                    )

        def proj_unit(b, pt):
            p0, psz = POS_T[pt]
            ob = wpool.tile([128, C], F32, tag="osb")
            for c0, csz in CCHUNK:
                ps = psum("psPJ")
                for kt in range(6):
                    nc.tensor.matmul(
                        ps[0:psz, 0:csz],
                        aoT[b][kt][:, p0 : p0 + psz],
                        wp[kt][:, c0 : c0 + csz],
                        start=(kt == 0),
                        stop=(kt == 5),
                    )
                nc.vector.tensor_add(
                    ob[0:psz, c0 : c0 + csz], ps[0:psz, 0:csz], pb_bc[0:psz, c0 : c0 + csz]
                )
            nc.sync.dma_start(out_d[b, p0 : p0 + psz, :], ob[0:psz, :])

        def qk_pool_unit(b, m):
            q_unit(b, m)
            if m < 6:
                pool_ct(b, m)

        def qkv_units(b):
            units = []
            for m in range(12):
                units.append(lambda m=m: qk_pool_unit(b, m))
            for pt in range(10):
                units.append(lambda pt=pt: v_unit(b, pt))
            return units

        # ---- schedule ----
        load_x(0)
        load_x(1)
        for u in qkv_units(0):
            u()

        for b in range(NB):
            fill = list(qkv_units(b + 1)) if b + 1 < NB else []
            if b + 2 < NB:
                load_x(b + 2)
            fi = 0

            def take(n):
                nonlocal fi
                for _ in range(n):
                    if fi < len(fill):
                        fill[fi]()
                        fi += 1

            aoT[b] = [
                hpool.tile([128, N], BF, tag=f"ao{i}", name=f"ao{i}", bufs=1)
                for i in range(6)
            ]
            for p2 in range(6):
                st = {}
                pair_scores(b, p2, st)
                take(1)
                pair_pv1(b, p2, st)
                pair_transp(b, p2, st)
                take(1)
                pair_pv2(b, p2, st)
                take(1)
                pair_pv3(b, p2, st)
            take(len(fill))
            for pt in range(10):
                proj_unit(b, pt)

    nc.compile()
    return nc


def tc_ctx(nc):
    from contextlib import contextmanager

    @contextmanager
    def ctx():
        with tile.TileContext(nc) as tc, nc.allow_low_precision(reason="attn bf16"):
            with (
                tc.tile_pool(name="const", bufs=1) as cpool,
                tc.tile_pool(name="work", bufs=2) as wpool,
                tc.tile_pool(name="hold", bufs=1) as hpool,
                tc.tile_pool(name="psum", bufs=8, space="PSUM") as ppool,
            ):
                yield tc, cpool, wpool, hpool, ppool

    return ctx()


_PROGRAM = None


def _get_program():
    global _PROGRAM
    if _PROGRAM is None:
        _PROGRAM = build_program()
    return _PROGRAM


def _prep_inputs(x, qkv_w, qkv_b, proj_w, proj_b):
    bf = ml_dtypes.bfloat16
    x = np.asarray(x, dtype=np.float32)
    xT = np.ascontiguousarray(x.transpose(0, 2, 1)).astype(bf)  # [B, C, N]
    wqkT = np.ascontiguousarray(np.asarray(qkv_w, dtype=np.float32).T).astype(bf)
    wpjT = np.ascontiguousarray(np.asarray(proj_w, dtype=np.float32).T).astype(bf)
    bqk = np.asarray(qkv_b, dtype=np.float32).reshape(1, -1).astype(bf)
    bqkp = np.ascontiguousarray(
        np.asarray(qkv_b, dtype=np.float32)[: 2 * 768].reshape(12, 128).T
    ).astype(np.float32)
    bpj = np.asarray(proj_b, dtype=np.float32).reshape(1, -1).astype(bf)
    in_maps = []
    for c in range(N_CORES):
        in_maps.append(
            {
                "xT": np.ascontiguousarray(xT[c * NB : (c + 1) * NB]),
                "wqkT": wqkT,
                "wpjT": wpjT,
                "bqk": bqk,
                "bqkp": bqkp,
                "bpj": bpj,
            }
        )
    return in_maps


def kernel(x, qkv_w, qkv_b, proj_w, proj_b, t_h=14, t_w=14, s_h=28, s_w=28, **kw):
    nc = _get_program()
    in_maps = _prep_inputs(x, qkv_w, qkv_b, proj_w, proj_b)
    res = bass_utils.run_bass_kernel_spmd(nc, in_maps, core_ids=list(range(N_CORES)))
    out = np.concatenate([res.results[c]["out"] for c in range(N_CORES)], axis=0)
    return out.astype(np.float32)


if __name__ == "__main__":
    build_program()
    print("program built OK")


# revision 18
# speedup vs baseline: 1.5505x; 1.0152x over previous
"""AgentAttention Trainium2 kernel: 8-core data-parallel over batch.

v2: cross-batch software pipeline. The TRN2 PE runs at 1.2GHz until it has
been continuously busy for ~3us, then 2.4GHz. The v1 kernel ran each batch's
attention stages serially, so the PE idled at every tensor<->scalar/vector
handoff and spent the whole attention phase at half clock. Here the dense
qkv GEMM units of batch b+1 are interleaved as fillers between the
dependency hops of batch b's attention, keeping the PE stream gap-free.

Layouts (per core, 4 batches):
  xT      [4, 768, 1176] bf16  (c-major x)
  qkT     c-major q,k: 12 sbuf tiles [128, 1176] (tiles 0-5 = q, 6-11 = k)
  v_ext   pos-major v with per-head ones column (col 64): 10 tiles [128, 12*65]
  agT     pooled agent tokens (sums over 4x4 blocks), c-major [128, 49] x6
  aoT     c-major attention output (bf16) [128, 1176] x6 -> proj -> out
Matmuls bf16, fp32 psum (uniform pool of 8 one-bank tiles [128,512]).
Softmax scale folded into ACT exp scale (0.125 stage1; 0.125/16 stages 2/3
-- agent tokens are pooled SUMS). qk bias via per-partition tensor_scalar.
"""

import sys

sys.path.insert(0, "/opt/trn_rl_repo")

import numpy as np
import ml_dtypes

import concourse.bass as bass
import concourse.mybir as mybir
import concourse.tile as tile
from concourse import bacc, bass_utils
from concourse.masks import make_identity

BF = mybir.dt.bfloat16
F32 = mybir.dt.float32
AF = mybir.ActivationFunctionType

N_CORES = 8
B, N, C = 32, 1176, 768
NB = B // N_CORES
H, HD = 12, 64
N_MT, N_S = 392, 784
A = 49
SCALE1 = 0.125
SCALE23 = 0.125 / 16.0

POS_T = [(pt * 128, min(128, N - pt * 128)) for pt in range(10)]
KEY1_T = [(0, 128), (128, 128), (256, 128), (384, 8)]
NCHUNK = [(0, 392), (392, 392), (784, 392)]
CCHUNK = [(0, 512), (512, 256)]
TSP = 116  # transpose chunk col spacing (>=113, even)


def build_program():
    nc = bacc.Bacc("TRN2", debug=False, num_devices=N_CORES)

    xT_d = nc.dram_tensor("xT", [NB, C, N], BF, kind="ExternalInput").ap()
    wqkT_d = nc.dram_tensor("wqkT", [C, 3 * C], BF, kind="ExternalInput").ap()
    wpjT_d = nc.dram_tensor("wpjT", [C, C], BF, kind="ExternalInput").ap()
    bqk_d = nc.dram_tensor("bqk", [1, 3 * C], BF, kind="ExternalInput").ap()
    bqkp_d = nc.dram_tensor("bqkp", [128, 12], F32, kind="ExternalInput").ap()
    bpj_d = nc.dram_tensor("bpj", [1, C], BF, kind="ExternalInput").ap()
    out_d = nc.dram_tensor("out", [NB, N, C], F32, kind="ExternalOutput").ap()

    with tc_ctx(nc) as (tc, cpool, wpool, hpool, ppool):
        # ---- one-time constants/weights ----
        wq = [
            cpool.tile([128, 3 * C], BF, tag=f"wq{i}", name=f"wq{i}") for i in range(6)
        ]
        wp = [cpool.tile([128, C], BF, tag=f"wp{i}", name=f"wp{i}") for i in range(6)]
        for i in range(6):
            nc.sync.dma_start(wq[i][:], wqkT_d[128 * i : 128 * (i + 1), :])
            nc.sync.dma_start(wp[i][:], wpjT_d[128 * i : 128 * (i + 1), :])
        sb_bqk = cpool.tile([1, 3 * C], BF, tag="bqk")
        nc.sync.dma_start(sb_bqk[:], bqk_d[:])
        bqkp = cpool.tile([128, 12], F32, tag="bqkp")
        nc.sync.dma_start(bqkp[:], bqkp_d[:])
        vb_bc = cpool.tile([128, C], BF, tag="vb_bc")
        nc.gpsimd.partition_broadcast(vb_bc[:], sb_bqk[0:1, 2 * C : 3 * C])
        bpjf = cpool.tile([1, C], BF, tag="bpjf", name="bpjf")
        nc.sync.dma_start(bpjf[:], bpj_d[:])
        pb_bc = cpool.tile([128, C], BF, tag="pb_bc")
        nc.gpsimd.partition_broadcast(pb_bc[:], bpjf[0:1, :])
        ident = cpool.tile([128, 128], BF, tag="ident")
        make_identity(nc, ident[:])

        # per-batch tile handles (rotated via tags, bufs=2)
        xT = {}
        qkT = {}
        v_ext = {}
        agT = {}
        aoT = {}

        def psum(name):
            return ppool.tile([128, 512], F32, tag="P", name=name, bufs=8)

        def load_x(b):
            xT[b] = [
                hpool.tile([128, N], BF, tag=f"xT{i}", name=f"xT{i}", bufs=2)
                for i in range(6)
            ]
            eng = nc.scalar if b == 0 else nc.sync
            for i in range(6):
                eng.dma_start(xT[b][i][:], xT_d[b, 128 * i : 128 * (i + 1), :])

        def q_unit(b, m):
            # qkT[m] c-major [128, 1176] for q (m<6) / k (m>=6) rows
            if m == 0:
                qkT[b] = [None] * 12
            t = hpool.tile([128, N], BF, tag=f"qkT{m}", name=f"qkT{m}", bufs=2)
            qkT[b][m] = t
            for n0, nsz in NCHUNK:
                ps = psum("psQ")
                for kt in range(6):
                    nc.tensor.matmul(
                        ps[:, 0:nsz],
                        wq[kt][:, 128 * m : 128 * (m + 1)],
                        xT[b][kt][:, n0 : n0 + nsz],
                        start=(kt == 0),
                        stop=(kt == 5),
                    )
                nc.vector.tensor_scalar_add(
                    t[:, n0 : n0 + nsz], ps[:, 0:nsz], bqkp[:, m : m + 1]
                )

        def v_unit(b, pt):
            # pos-major v_ext [psz, 12*65] with ones col at 64 of each head
            p0, psz = POS_T[pt]
            if pt == 0:
                v_ext[b] = [None] * 10
            vt = hpool.tile([128, H * 65], BF, tag=f"vx{pt}", name=f"vx{pt}", bufs=2)
            v_ext[b][pt] = vt
            if b < 2:
                # two rotation slots; evac only writes the 64 v columns, so
                # ones persist across later batches
                nc.vector.memset(
                    vt[:].rearrange("p (h e) -> p h e", e=65)[:, :, 64:65], 1.0
                )
            for ci, (c0, csz) in enumerate(CCHUNK):
                ps = psum("psV")
                for kt in range(6):
                    nc.tensor.matmul(
                        ps[0:psz, 0:csz],
                        xT[b][kt][:, p0 : p0 + psz],
                        wq[kt][:, 2 * C + c0 : 2 * C + c0 + csz],
                        start=(kt == 0),
                        stop=(kt == 5),
                    )
                nh = csz // 64
                h0 = c0 // 64
                nc.vector.tensor_add(
                    vt[0:psz].rearrange("p (h e) -> p h e", e=65)[
                        :, h0 : h0 + nh, 0:64
                    ],
                    ps[0:psz, 0:csz].rearrange("p (h d) -> p h d", d=64),
                    vb_bc[0:psz, c0 : c0 + csz].rearrange("p (h d) -> p h d", d=64),
                )

        def pool_ct(b, ct):
            # sum 4x4 blocks of q_s -> agT (c-major). On VECTOR: gpsimd must
            # stay broadcast-only (lib swaps + in-order blocking starve the
            # norm-chain broadcasts otherwise)
            if ct == 0:
                agT[b] = []
            t1 = wpool.tile([128, 196], F32, tag="t1", bufs=1)
            qs = qkT[b][ct][:, N_MT:N]  # [128, 784], idx = i*28 + aj*4 + dj
            q4 = qs.rearrange("p (x dj) -> p x dj", dj=4)
            nc.vector.tensor_add(t1[:, 0:196], q4[:, :, 0:1], q4[:, :, 1:2])
            nc.vector.tensor_add(t1[:, 0:196], t1[:, 0:196], q4[:, :, 2:3])
            nc.vector.tensor_add(t1[:, 0:196], t1[:, 0:196], q4[:, :, 3:4])
            ag = hpool.tile([128, A], BF, tag=f"ag{ct}", name=f"ag{ct}", bufs=2)
            agT[b].append(ag)
            # t1 idx = 28*ai + 7*di + aj -> view (ai, aj, di)
            t4 = t1[:, 0:196].rearrange("p (ai di aj) -> p ai aj di", ai=7, di=4)
            t2 = wpool.tile([128, A], F32, tag="t2")
            nc.vector.tensor_add(t2[:, 0:A], t4[:, :, :, 0:1], t4[:, :, :, 1:2])
            nc.vector.tensor_add(t2[:, 0:A], t2[:, 0:A], t4[:, :, :, 2:3])
            nc.vector.tensor_add(ag[:, 0:A], t2[:, 0:A], t4[:, :, :, 3:4])

        def norm_chain(pv, dst):
            # dst = pv[0:64] / pv[64] (per free-dim query), pv is psum
            se = wpool.tile([1, 392], F32, tag="se", bufs=1)
            nc.vector.tensor_copy(se[:, 0:392], pv[64:65, 0:392])
            rc = wpool.tile([1, 392], F32, tag="rc", bufs=2)
            nc.vector.reciprocal_approx_fast(out=rc[:, 0:392], in_=se[:, 0:392])
            bc = wpool.tile([64, 392], F32, tag="bc", bufs=2)
            nc.gpsimd.partition_broadcast(bc[:], rc[0:1, 0:392])
            nc.vector.tensor_mul(dst, pv[0:64, 0:392], bc[:])

        # ---- attention for one head pair, split into schedulable chunks ----
        def pair_scores(b, p2, st):
            qt = p2
            # stage 1 scores first: [keys, queries] per head over 4 key chunks.
            # Claim order matches exp (= psum evacuation) order so the 8-bank
            # rotation never waits, and e1 (pv1's dep) is computed earliest.
            st["s1"] = []
            for hp in range(2):
                qo = 64 * hp
                chunks = []
                st["s1"].append(chunks)
                for k0, ksz in KEY1_T:
                    ps = psum("psS1")
                    chunks.append(ps)
                    nc.tensor.matmul(
                        ps[0:ksz, 0:392],
                        qkT[b][6 + qt][qo : qo + 64, k0 : k0 + ksz],
                        qkT[b][qt][qo : qo + 64, 0:N_MT],
                        start=True,
                        stop=True,
                    )
            # stage 2 scores: [49x2 packed, keys] over 3 chunks
            st["s2"] = []
            for n0, nsz in NCHUNK:
                ps = psum("psS2")
                st["s2"].append(ps)
                for hp in range(2):
                    qo = 64 * hp
                    nc.tensor.matmul(
                        ps[qo : qo + 49, 0:nsz],
                        agT[b][qt][qo : qo + 64, 0:A],
                        qkT[b][6 + qt][qo : qo + 64, n0 : n0 + nsz],
                        start=True,
                        stop=True,
                    )
            # stage 3 scores: [49x2 packed (agents), queries] over 2 chunks
            st["s3"] = []
            for cc in range(2):
                ps = psum("psS3")
                st["s3"].append(ps)
                for hp in range(2):
                    qo = 64 * hp
                    nc.tensor.matmul(
                        ps[qo : qo + 49, 0:392],
                        agT[b][qt][qo : qo + 64, 0:A],
                        qkT[b][qt][qo : qo + 64, N_MT + 392 * cc : N_MT + 392 * (cc + 1)],
                        start=True,
                        stop=True,
                    )
            # exps (scalar engine) in the same order as the score claims
            st["e1"] = []
            for hp in range(2):
                e1s = []
                st["e1"].append(e1s)
                for j, (k0, ksz) in enumerate(KEY1_T):
                    e1 = wpool.tile([128, 392], BF, tag="e1", name="e1", bufs=8)
                    e1s.append(e1)
                    nc.scalar.activation(
                        e1[0:ksz, 0:392],
                        st["s1"][hp][j][0:ksz, 0:392],
                        AF.Exp,
                        scale=SCALE1,
                    )
            e2 = wpool.tile([128, N], BF, tag="e2")
            st["e2"] = e2
            for j, (n0, nsz) in enumerate(NCHUNK):
                nc.scalar.activation(
                    e2[0:113, n0 : n0 + nsz],
                    st["s2"][j][0:113, 0:nsz],
                    AF.Exp,
                    scale=SCALE23,
                )
            st["e3"] = []
            for cc in range(2):
                e3 = wpool.tile([128, 392], BF, tag="e3", name="e3", bufs=2)
                st["e3"].append(e3)
                nc.scalar.activation(
                    e3[0:113, 0:392], st["s3"][cc][0:113, 0:392], AF.Exp, scale=SCALE23
                )

        def pair_pv1_mm(b, p2, st):
            st["pv1"] = []
            for hp in range(2):
                pv = psum("psPV1")
                st["pv1"].append(pv)
                for j, (k0, ksz) in enumerate(KEY1_T):
                    nc.tensor.matmul(
                        pv[0:65, 0:392],
                        v_ext[b][j][0:ksz, 65 * (2 * p2 + hp) : 65 * (2 * p2 + hp) + 65],
                        st["e1"][hp][j][0:ksz, 0:392],
                        start=(j == 0),
                        stop=(j == 3),
                    )

        def pair_pv1_norm(b, p2, st):
            qt = p2
            for hp in range(2):
                qo = 64 * hp
                norm_chain(st["pv1"][hp], aoT[b][qt][qo : qo + 64, 0:N_MT])

        def pair_transp(b, p2, st):
            # [113, keys] -> [keys, 113] in 10 chunks, via identity matmul
            st["eT"] = []
            for half in range(2):
                trp = ppool.tile([128, 5 * TSP], BF, tag="P", name="psTr", bufs=8)
                for kk in range(5):
                    kt = 5 * half + kk
                    p0, psz = POS_T[kt]
                    nc.tensor.transpose(
                        trp[0:psz, TSP * kk : TSP * kk + 113],
                        st["e2"][0:113, p0 : p0 + psz],
                        ident[0:113, 0:113],
                    )
                eT = wpool.tile([128, 5 * TSP], BF, tag="e2T", bufs=2)
                st["eT"].append(eT)
                nc.vector.tensor_copy(eT[:, 0 : 5 * TSP], trp[:, 0 : 5 * TSP])

        def pair_pv2(b, p2, st):
            pv2 = psum("psPV2")
            for hp in range(2):
                h = 2 * p2 + hp
                o = 64 * hp
                for kt, (p0, psz) in enumerate(POS_T):
                    eT = st["eT"][kt // 5]
                    cof = TSP * (kt % 5) + 64 * hp
                    nc.tensor.matmul(
                        pv2[o : o + 49, 0:65],
                        eT[0:psz, cof : cof + 49],
                        v_ext[b][kt][0:psz, 65 * h : 65 * h + 65],
                        start=(kt == 0),
                        stop=(kt == 9),
                    )
            av = wpool.tile([128, 65], BF, tag="avx", bufs=2)
            st["av"] = av
            nc.vector.memset(av[0:113, 64:65], 1.0)
            avr = wpool.tile([128, 1], F32, tag="avr", bufs=2)
            nc.vector.reciprocal(avr[0:113, 0:1], pv2[0:113, 64:65])
            nc.vector.tensor_scalar_mul(
                av[0:113, 0:64], pv2[0:113, 0:64], avr[0:113, 0:1]
            )

        def pair_pv3(b, p2, st):
            qt = p2
            for hp in range(2):
                qo = 64 * hp
                for cc in range(2):
                    pv = psum("psPV3")
                    nc.tensor.matmul(
                        pv[0:65, 0:392],
                        st["av"][64 * hp : 64 * hp + 49, 0:65],
                        st["e3"][cc][64 * hp : 64 * hp + 49, 0:392],
                        start=True,
                        stop=True,
                    )
                    norm_chain(
                        pv,
                        aoT[b][qt][qo : qo + 64, N_MT + 392 * cc : N_MT + 392 * (cc + 1)],
                    )

        def proj_unit(b, pt):
            p0, psz = POS_T[pt]
            ob = wpool.tile([128, C], F32, tag="osb")
            for c0, csz in CCHUNK:
                ps = psum("psPJ")
                for kt in range(6):
                    nc.tensor.matmul(
                        ps[0:psz, 0:csz],
                        aoT[b][kt][:, p0 : p0 + psz],
                        wp[kt][:, c0 : c0 + csz],
                        start=(kt == 0),
                        stop=(kt == 5),
                    )
                nc.vector.tensor_add(
                    ob[0:psz, c0 : c0 + csz], ps[0:psz, 0:csz], pb_bc[0:psz, c0 : c0 + csz]
                )
            nc.sync.dma_start(out_d[b, p0 : p0 + psz, :], ob[0:psz, :])

        def qk_pool_unit(b, m):
            q_unit(b, m)
            if m < 6:
                pool_ct(b, m)

        def qkv_units(b):
            units = []
            for m in range(12):
                units.append(lambda m=m: qk_pool_unit(b, m))
            for pt in range(10):
                units.append(lambda pt=pt: v_unit(b, pt))
            return units

        # ---- schedule ----
        load_x(0)
        load_x(1)
        for u in qkv_units(0):
            u()

        for b in range(NB):
            fill = list(qkv_units(b + 1)) if b + 1 < NB else []
            if b + 2 < NB:
                load_x(b + 2)
            fi = 0

            def take(n):
                nonlocal fi
                for _ in range(n):
                    if fi < len(fill):
                        fill[fi]()
                        fi += 1

            aoT[b] = [
                hpool.tile([128, N], BF, tag=f"ao{i}", name=f"ao{i}", bufs=1)
                for i in range(6)
            ]
            for p2 in range(6):
                st = {}
                pair_scores(b, p2, st)
                take(1)
                pair_pv1_mm(b, p2, st)
                pair_transp(b, p2, st)
                pair_pv1_norm(b, p2, st)
                if p2 < 5:
                    take(1)
                    pair_pv2(b, p2, st)
                    take(1)
                    pair_pv3(b, p2, st)
                else:
                    # last pair: proj tiles 0-2 (x_mt region, stage1-only dep)
                    # serve as the fillers for its tail
                    take(1)
                    proj_unit(b, 0)
                    pair_pv2(b, p2, st)
                    proj_unit(b, 1)
                    pair_pv3(b, p2, st)
                    proj_unit(b, 2)
            take(len(fill))
            for pt in range(3, 10):
                proj_unit(b, pt)

    nc.compile()
    return nc


def tc_ctx(nc):
    from contextlib import contextmanager

    @contextmanager
    def ctx():
        with tile.TileContext(nc) as tc, nc.allow_low_precision(reason="attn bf16"):
            with (
                tc.tile_pool(name="const", bufs=1) as cpool,
                tc.tile_pool(name="work", bufs=2) as wpool,
                tc.tile_pool(name="hold", bufs=1) as hpool,
                tc.tile_pool(name="psum", bufs=8, space="PSUM") as ppool,
            ):
                yield tc, cpool, wpool, hpool, ppool

    return ctx()


_PROGRAM = None


def _get_program():
    global _PROGRAM
    if _PROGRAM is None:
        _PROGRAM = build_program()
    return _PROGRAM


def _prep_inputs(x, qkv_w, qkv_b, proj_w, proj_b):
    bf = ml_dtypes.bfloat16
    x = np.asarray(x, dtype=np.float32)
    xT = np.ascontiguousarray(x.transpose(0, 2, 1)).astype(bf)  # [B, C, N]
    wqkT = np.ascontiguousarray(np.asarray(qkv_w, dtype=np.float32).T).astype(bf)
    wpjT = np.ascontiguousarray(np.asarray(proj_w, dtype=np.float32).T).astype(bf)
    bqk = np.asarray(qkv_b, dtype=np.float32).reshape(1, -1).astype(bf)
    bqkp = np.ascontiguousarray(
        np.asarray(qkv_b, dtype=np.float32)[: 2 * 768].reshape(12, 128).T
    ).astype(np.float32)
    bpj = np.asarray(proj_b, dtype=np.float32).reshape(1, -1).astype(bf)
    in_maps = []
    for c in range(N_CORES):
        in_maps.append(
            {
                "xT": np.ascontiguousarray(xT[c * NB : (c + 1) * NB]),
                "wqkT": wqkT,
                "wpjT": wpjT,
                "bqk": bqk,
                "bqkp": bqkp,
                "bpj": bpj,
            }
        )
    return in_maps


def kernel(x, qkv_w, qkv_b, proj_w, proj_b, t_h=14, t_w=14, s_h=28, s_w=28, **kw):
    nc = _get_program()
    in_maps = _prep_inputs(x, qkv_w, qkv_b, proj_w, proj_b)
    res = bass_utils.run_bass_kernel_spmd(nc, in_maps, core_ids=list(range(N_CORES)))
    out = np.concatenate([res.results[c]["out"] for c in range(N_CORES)], axis=0)
    return out.astype(np.float32)


if __name__ == "__main__":
    build_program()
    print("program built OK")


# revision 27
# speedup vs baseline: 1.5888x; 1.0247x over previous
"""AgentAttention Trainium2 kernel: 8-core data-parallel over batch.

v2: cross-batch software pipeline. The TRN2 PE runs at 1.2GHz until it has
been continuously busy for ~3us, then 2.4GHz. The v1 kernel ran each batch's
attention stages serially, so the PE idled at every tensor<->scalar/vector
handoff and spent the whole attention phase at half clock. Here the dense
qkv GEMM units of batch b+1 are interleaved as fillers between the
dependency hops of batch b's attention, keeping the PE stream gap-free.

Layouts (per core, 4 batches):
  xT      [4, 768, 1176] bf16  (c-major x)
  qkT     c-major q,k: 12 sbuf tiles [128, 1176] (tiles 0-5 = q, 6-11 = k)
  v_ext   pos-major v with per-head ones column (col 64): 10 tiles [128, 12*65]
  agT     pooled agent tokens (sums over 4x4 blocks), c-major [128, 49] x6
  aoT     c-major attention output (bf16) [128, 1176] x6 -> proj -> out
Matmuls bf16, fp32 psum (uniform pool of 8 one-bank tiles [128,512]).
Softmax scale folded into ACT exp scale (0.125 stage1; 0.125/16 stages 2/3
-- agent tokens are pooled SUMS). qk bias via per-partition tensor_scalar.
"""

import sys

sys.path.insert(0, "/opt/trn_rl_repo")

import numpy as np
import ml_dtypes

import concourse.bass as bass
import concourse.mybir as mybir
import concourse.tile as tile
from concourse import bacc, bass_utils
from concourse.masks import make_identity

BF = mybir.dt.bfloat16
F32 = mybir.dt.float32
AF = mybir.ActivationFunctionType

N_CORES = 8
B, N, C = 32, 1176, 768
NB = B // N_CORES
H, HD = 12, 64
N_MT, N_S = 392, 784
A = 49
SCALE1 = 0.125
SCALE23 = 0.125 / 16.0

POS_T = [(pt * 128, min(128, N - pt * 128)) for pt in range(10)]
KEY1_T = [(0, 128), (128, 128), (256, 128), (384, 8)]
NCHUNK = [(0, 392), (392, 392), (784, 392)]
CCHUNK = [(0, 512), (512, 256)]
TSP = 116  # transpose chunk col spacing (>=113, even)


def build_program():
    nc = bacc.Bacc("TRN2", debug=False, num_devices=N_CORES)

    xT_d = nc.dram_tensor("xT", [NB, C, N], BF, kind="ExternalInput").ap()
    wqkT_d = nc.dram_tensor("wqkT", [C, 3 * C], BF, kind="ExternalInput").ap()
    wpjT_d = nc.dram_tensor("wpjT", [C, C], BF, kind="ExternalInput").ap()
    vbb_d = nc.dram_tensor("vbb", [128, C], BF, kind="ExternalInput").ap()
    bqkp_d = nc.dram_tensor("bqkp", [128, 12], F32, kind="ExternalInput").ap()
    pbb_d = nc.dram_tensor("pbb", [128, C], BF, kind="ExternalInput").ap()
    out_d = nc.dram_tensor("out", [NB, N, C], F32, kind="ExternalOutput").ap()

    with tc_ctx(nc) as (tc, cpool, wpool, hpool, ppool):
        # ---- one-time constants/weights ----
        wq = [
            cpool.tile([128, 3 * C], BF, tag=f"wq{i}", name=f"wq{i}") for i in range(6)
        ]
        wp = [cpool.tile([128, C], BF, tag=f"wp{i}", name=f"wp{i}") for i in range(6)]
        for i in range(6):
            nc.sync.dma_start(wq[i][:], wqkT_d[128 * i : 128 * (i + 1), :])
            nc.sync.dma_start(wp[i][:], wpjT_d[128 * i : 128 * (i + 1), :])
        bqkp = cpool.tile([128, 12], F32, tag="bqkp")
        nc.sync.dma_start(bqkp[:], bqkp_d[:])
        vb_bc = cpool.tile([128, C], BF, tag="vb_bc")
        nc.sync.dma_start(vb_bc[:], vbb_d[:])
        pb_bc = cpool.tile([128, C], BF, tag="pb_bc")
        nc.sync.dma_start(pb_bc[:], pbb_d[:])
        ident = cpool.tile([128, 128], BF, tag="ident")
        make_identity(nc, ident[:])

        # per-batch tile handles (rotated via tags, bufs=2)
        xT = {}
        qkT = {}
        v_ext = {}
        agT = {}
        aoT = {}

        def psum(name):
            return ppool.tile([128, 512], F32, tag="P", name=name, bufs=8)

        def load_x(b):
            xT[b] = [
                hpool.tile([128, N], BF, tag=f"xT{i}", name=f"xT{i}", bufs=2)
                for i in range(6)
            ]
            eng = nc.scalar if b == 0 else nc.sync
            for i in range(6):
                eng.dma_start(xT[b][i][:], xT_d[b, 128 * i : 128 * (i + 1), :])

        def q_unit(b, m):
            # qkT[m] c-major [128, 1176] for q (m<6) / k (m>=6) rows
            if m == 0:
                qkT[b] = [None] * 12
            t = hpool.tile([128, N], BF, tag=f"qkT{m}", name=f"qkT{m}", bufs=2)
            qkT[b][m] = t
            for n0, nsz in NCHUNK:
                ps = psum("psQ")
                for kt in range(6):
                    nc.tensor.matmul(
                        ps[:, 0:nsz],
                        wq[kt][:, 128 * m : 128 * (m + 1)],
                        xT[b][kt][:, n0 : n0 + nsz],
                        start=(kt == 0),
                        stop=(kt == 5),
                    )
                # evac on scalar engine: vector is the congested one
                nc.scalar.activation(
                    t[:, n0 : n0 + nsz],
                    ps[:, 0:nsz],
                    AF.Identity,
                    bias=bqkp[:, m : m + 1],
                )

        def v_unit(b, pt):
            # pos-major v_ext [psz, 12*65] with ones col at 64 of each head
            p0, psz = POS_T[pt]
            if pt == 0:
                v_ext[b] = [None] * 10
            vt = hpool.tile([128, H * 65], BF, tag=f"vx{pt}", name=f"vx{pt}", bufs=2)
            v_ext[b][pt] = vt
            if b < 2:
                # two rotation slots; evac only writes the 64 v columns, so
                # ones persist across later batches
                nc.vector.memset(
                    vt[:].rearrange("p (h e) -> p h e", e=65)[:, :, 64:65], 1.0
                )
            for ci, (c0, csz) in enumerate(CCHUNK):
                ps = psum("psV")
                for kt in range(6):
                    nc.tensor.matmul(
                        ps[0:psz, 0:csz],
                        xT[b][kt][:, p0 : p0 + psz],
                        wq[kt][:, 2 * C + c0 : 2 * C + c0 + csz],
                        start=(kt == 0),
                        stop=(kt == 5),
                    )
                nh = csz // 64
                h0 = c0 // 64
                nc.vector.tensor_add(
                    vt[0:psz].rearrange("p (h e) -> p h e", e=65)[
                        :, h0 : h0 + nh, 0:64
                    ],
                    ps[0:psz, 0:csz].rearrange("p (h d) -> p h d", d=64),
                    vb_bc[0:psz, c0 : c0 + csz].rearrange("p (h d) -> p h d", d=64),
                )

        def pool_ct(b, ct):
            # sum 4x4 blocks of q_s -> agT (c-major). On VECTOR: gpsimd must
            # stay broadcast-only (lib swaps + in-order blocking starve the
            # norm-chain broadcasts otherwise)
            if ct == 0:
                agT[b] = []
            t1 = wpool.tile([128, 196], F32, tag="t1", bufs=1)
            qs = qkT[b][ct][:, N_MT:N]  # [128, 784], idx = i*28 + aj*4 + dj
            q4 = qs.rearrange("p (x dj) -> p x dj", dj=4)
            nc.vector.tensor_add(t1[:, 0:196], q4[:, :, 0:1], q4[:, :, 1:2])
            nc.vector.tensor_add(t1[:, 0:196], t1[:, 0:196], q4[:, :, 2:3])
            nc.vector.tensor_add(t1[:, 0:196], t1[:, 0:196], q4[:, :, 3:4])
            ag = hpool.tile([128, A], BF, tag=f"ag{ct}", name=f"ag{ct}", bufs=2)
            agT[b].append(ag)
            # t1 idx = 28*ai + 7*di + aj -> view (ai, aj, di)
            t4 = t1[:, 0:196].rearrange("p (ai di aj) -> p ai aj di", ai=7, di=4)
            t2 = wpool.tile([128, A], F32, tag="t2")
            nc.vector.tensor_add(t2[:, 0:A], t4[:, :, :, 0:1], t4[:, :, :, 1:2])
            nc.vector.tensor_add(t2[:, 0:A], t2[:, 0:A], t4[:, :, :, 2:3])
            nc.vector.tensor_add(ag[:, 0:A], t2[:, 0:A], t4[:, :, :, 3:4])

        def norm_chain(pv, dst):
            # dst = pv[0:64] / pv[64] (per free-dim query), pv is psum
            se = wpool.tile([1, 392], F32, tag="se", bufs=1)
            nc.vector.tensor_copy(se[:, 0:392], pv[64:65, 0:392])
            rc = wpool.tile([1, 392], F32, tag="rc", bufs=2)
            nc.vector.reciprocal_approx_fast(out=rc[:, 0:392], in_=se[:, 0:392])
            bc = wpool.tile([64, 392], F32, tag="bc", bufs=2)
            nc.gpsimd.partition_broadcast(bc[:], rc[0:1, 0:392])
            nc.vector.tensor_mul(dst, pv[0:64, 0:392], bc[:])

        # ---- attention for one head pair, split into schedulable chunks ----
        def pair_scores(b, p2, st):
            qt = p2
            # stage 1 scores first: [keys, queries] per head over 4 key chunks.
            # Claim order matches exp (= psum evacuation) order so the 8-bank
            # rotation never waits, and e1 (pv1's dep) is computed earliest.
            st["s1"] = []
            for hp in range(2):
                qo = 64 * hp
                chunks = []
                st["s1"].append(chunks)
                for k0, ksz in KEY1_T:
                    ps = psum("psS1")
                    chunks.append(ps)
                    nc.tensor.matmul(
                        ps[0:ksz, 0:392],
                        qkT[b][6 + qt][qo : qo + 64, k0 : k0 + ksz],
                        qkT[b][qt][qo : qo + 64, 0:N_MT],
                        start=True,
                        stop=True,
                    )
            # stage 2 scores: [49x2 packed, keys] over 3 chunks
            st["s2"] = []
            for n0, nsz in NCHUNK:
                ps = psum("psS2")
                st["s2"].append(ps)
                for hp in range(2):
                    qo = 64 * hp
                    nc.tensor.matmul(
                        ps[qo : qo + 49, 0:nsz],
                        agT[b][qt][qo : qo + 64, 0:A],
                        qkT[b][6 + qt][qo : qo + 64, n0 : n0 + nsz],
                        start=True,
                        stop=True,
                    )
            # stage 3 scores: [49x2 packed (agents), queries] over 2 chunks
            st["s3"] = []
            for cc in range(2):
                ps = psum("psS3")
                st["s3"].append(ps)
                for hp in range(2):
                    qo = 64 * hp
                    nc.tensor.matmul(
                        ps[qo : qo + 49, 0:392],
                        agT[b][qt][qo : qo + 64, 0:A],
                        qkT[b][qt][qo : qo + 64, N_MT + 392 * cc : N_MT + 392 * (cc + 1)],
                        start=True,
                        stop=True,
                    )
            # exps (scalar engine) in the same order as the score claims
            st["e1"] = []
            for hp in range(2):
                e1s = []
                st["e1"].append(e1s)
                for j, (k0, ksz) in enumerate(KEY1_T):
                    e1 = wpool.tile([128, 392], BF, tag="e1", name="e1", bufs=8)
                    e1s.append(e1)
                    nc.scalar.activation(
                        e1[0:ksz, 0:392],
                        st["s1"][hp][j][0:ksz, 0:392],
                        AF.Exp,
                        scale=SCALE1,
                    )
            e2 = wpool.tile([128, N], BF, tag="e2")
            st["e2"] = e2
            for j, (n0, nsz) in enumerate(NCHUNK):
                nc.scalar.activation(
                    e2[0:113, n0 : n0 + nsz],
                    st["s2"][j][0:113, 0:nsz],
                    AF.Exp,
                    scale=SCALE23,
                )
            st["e3"] = []
            for cc in range(2):
                e3 = wpool.tile([128, 392], BF, tag="e3", name="e3", bufs=2)
                st["e3"].append(e3)
                nc.scalar.activation(
                    e3[0:113, 0:392], st["s3"][cc][0:113, 0:392], AF.Exp, scale=SCALE23
                )

        def pair_pv1_mm(b, p2, st):
            st["pv1"] = []
            for hp in range(2):
                pv = psum("psPV1")
                st["pv1"].append(pv)
                for j, (k0, ksz) in enumerate(KEY1_T):
                    nc.tensor.matmul(
                        pv[0:65, 0:392],
                        v_ext[b][j][0:ksz, 65 * (2 * p2 + hp) : 65 * (2 * p2 + hp) + 65],
                        st["e1"][hp][j][0:ksz, 0:392],
                        start=(j == 0),
                        stop=(j == 3),
                    )

        def pair_pv1_norm(b, p2, st):
            qt = p2
            for hp in range(2):
                qo = 64 * hp
                norm_chain(st["pv1"][hp], aoT[b][qt][qo : qo + 64, 0:N_MT])

        def pair_transp(b, p2, st):
            # [113, keys] -> [keys, 113] in 10 chunks, via identity matmul
            st["eT"] = []
            for half in range(2):
                trp = ppool.tile([128, 5 * TSP], BF, tag="P", name="psTr", bufs=8)
                for kk in range(5):
                    kt = 5 * half + kk
                    p0, psz = POS_T[kt]
                    nc.tensor.transpose(
                        trp[0:psz, TSP * kk : TSP * kk + 113],
                        st["e2"][0:113, p0 : p0 + psz],
                        ident[0:113, 0:113],
                    )
                eT = wpool.tile([128, 5 * TSP], BF, tag="e2T", bufs=2)
                st["eT"].append(eT)
                nc.scalar.activation(eT[:, 0 : 5 * TSP], trp[:, 0 : 5 * TSP], AF.Copy)

        def pair_pv2(b, p2, st):
            # both heads per matmul: lhsT = full transposed tile (garbage rows
            # 49:63 only pollute unused output rows), rhs = 129-wide v_ext
            # slice [v_h0 | ones | v_h1]; the ones col yields both heads'
            # sumexp at out col 64. 10 matmuls instead of 20.
            pv2 = psum("psPV2")
            for kt, (p0, psz) in enumerate(POS_T):
                eT = st["eT"][kt // 5]
                cof = TSP * (kt % 5)
                nc.tensor.matmul(
                    pv2[0:113, 0:129],
                    eT[0:psz, cof : cof + 113],
                    v_ext[b][kt][0:psz, 130 * p2 : 130 * p2 + 129],
                    start=(kt == 0),
                    stop=(kt == 9),
                )
            av = wpool.tile([128, 65], BF, tag="avx", bufs=2)
            st["av"] = av
            nc.vector.memset(av[0:113, 64:65], 1.0)
            avr = wpool.tile([128, 1], F32, tag="avr", bufs=2)
            nc.vector.reciprocal(avr[0:113, 0:1], pv2[0:113, 64:65])
            nc.vector.tensor_scalar_mul(av[0:49, 0:64], pv2[0:49, 0:64], avr[0:49, 0:1])
            nc.vector.tensor_scalar_mul(
                av[64:113, 0:64], pv2[64:113, 65:129], avr[64:113, 0:1]
            )

        def pair_pv3(b, p2, st):
            qt = p2
            for hp in range(2):
                qo = 64 * hp
                for cc in range(2):
                    pv = psum("psPV3")
                    nc.tensor.matmul(
                        pv[0:65, 0:392],
                        st["av"][64 * hp : 64 * hp + 49, 0:65],
                        st["e3"][cc][64 * hp : 64 * hp + 49, 0:392],
                        start=True,
                        stop=True,
                    )
                    norm_chain(
                        pv,
                        aoT[b][qt][qo : qo + 64, N_MT + 392 * cc : N_MT + 392 * (cc + 1)],
                    )

        def proj_unit(b, pt):
            p0, psz = POS_T[pt]
            ob = wpool.tile([128, C], F32, tag="osb")
            for c0, csz in CCHUNK:
                ps = psum("psPJ")
                for kt in range(6):
                    nc.tensor.matmul(
                        ps[0:psz, 0:csz],
                        aoT[b][kt][:, p0 : p0 + psz],
                        wp[kt][:, c0 : c0 + csz],
                        start=(kt == 0),
                        stop=(kt == 5),
                    )
                nc.vector.tensor_add(
                    ob[0:psz, c0 : c0 + csz], ps[0:psz, 0:csz], pb_bc[0:psz, c0 : c0 + csz]
                )
            nc.sync.dma_start(out_d[b, p0 : p0 + psz, :], ob[0:psz, :])

        def qk_pool_unit(b, m):
            q_unit(b, m)
            if m < 6:
                pool_ct(b, m)

        def qkv_units(b):
            units = []
            for m in range(12):
                units.append(lambda m=m: qk_pool_unit(b, m))
            for pt in range(10):
                units.append(lambda pt=pt: v_unit(b, pt))
            return units

        # ---- schedule ----
        load_x(0)
        load_x(1)
        for u in qkv_units(0):
            u()

        for b in range(NB):
            fill = list(qkv_units(b + 1)) if b + 1 < NB else []
            if b + 2 < NB:
                load_x(b + 2)
            fi = 0

            def take(n):
                nonlocal fi
                for _ in range(n):
                    if fi < len(fill):
                        fill[fi]()
                        fi += 1

            aoT[b] = [
                hpool.tile([128, N], BF, tag=f"ao{i}", name=f"ao{i}", bufs=1)
                for i in range(6)
            ]
            if fill:
                for p2 in range(6):
                    st = {}
                    pair_scores(b, p2, st)
                    take(1)
                    pair_pv1_mm(b, p2, st)
                    pair_transp(b, p2, st)
                    pair_pv1_norm(b, p2, st)
                    if p2 < 5:
                        take(1)
                        pair_pv2(b, p2, st)
                        take(1)
                        pair_pv3(b, p2, st)
                    else:
                        # last pair: proj tiles 0-2 (x_mt region, stage1-only
                        # dep) serve as the fillers for its tail
                        take(1)
                        proj_unit(b, 0)
                        pair_pv2(b, p2, st)
                        proj_unit(b, 1)
                        pair_pv3(b, p2, st)
                        proj_unit(b, 2)
                take(len(fill))
                for pt in range(3, 10):
                    proj_unit(b, pt)
            else:
                # last batch: no qkv fillers left. Run the pairs as a 2-deep
                # software pipeline so each pair's dependency gaps are filled
                # by the neighbouring pair's independent matmuls. Verified to
                # fit e1(8)/e2/e2T/e3/av bufs: at most 2 pairs in flight, and
                # every reuse's consumer is emitted before the reclaiming
                # producer.
                sts = [dict() for _ in range(6)]
                pair_scores(b, 0, sts[0])
                pair_scores(b, 1, sts[1])
                for p2 in range(6):
                    pair_pv1_mm(b, p2, sts[p2])
                    pair_transp(b, p2, sts[p2])
                    pair_pv1_norm(b, p2, sts[p2])
                    if p2 + 2 < 6:
                        pair_scores(b, p2 + 2, sts[p2 + 2])
                    pair_pv2(b, p2, sts[p2])
                    if p2 == 5:
                        proj_unit(b, 0)
                    pair_pv3(b, p2, sts[p2])
                for pt in range(1, 10):
                    proj_unit(b, pt)

    nc.compile()
    return nc


def tc_ctx(nc):
    from contextlib import contextmanager

    @contextmanager
    def ctx():
        with tile.TileContext(nc) as tc, nc.allow_low_precision(reason="attn bf16"):
            with (
                tc.tile_pool(name="const", bufs=1) as cpool,
                tc.tile_pool(name="work", bufs=2) as wpool,
                tc.tile_pool(name="hold", bufs=1) as hpool,
                tc.tile_pool(name="psum", bufs=8, space="PSUM") as ppool,
            ):
                yield tc, cpool, wpool, hpool, ppool

    return ctx()


_PROGRAM = None


def _get_program():
    global _PROGRAM
    if _PROGRAM is None:
        _PROGRAM = build_program()
    return _PROGRAM


def _prep_inputs(x, qkv_w, qkv_b, proj_w, proj_b):
    bf = ml_dtypes.bfloat16
    x = np.asarray(x, dtype=np.float32)
    xT = np.ascontiguousarray(x.transpose(0, 2, 1)).astype(bf)  # [B, C, N]
    wqkT = np.ascontiguousarray(np.asarray(qkv_w, dtype=np.float32).T).astype(bf)
    wpjT = np.ascontiguousarray(np.asarray(proj_w, dtype=np.float32).T).astype(bf)
    qb = np.asarray(qkv_b, dtype=np.float32)
    vbb = np.broadcast_to(qb[2 * 768 :].astype(bf), (128, 768)).copy()
    bqkp = np.ascontiguousarray(qb[: 2 * 768].reshape(12, 128).T).astype(np.float32)
    pbb = np.broadcast_to(
        np.asarray(proj_b, dtype=np.float32).astype(bf), (128, 768)
    ).copy()
    in_maps = []
    for c in range(N_CORES):
        in_maps.append(
            {
                "xT": np.ascontiguousarray(xT[c * NB : (c + 1) * NB]),
                "wqkT": wqkT,
                "wpjT": wpjT,
                "vbb": vbb,
                "bqkp": bqkp,
                "pbb": pbb,
            }
        )
    return in_maps


def kernel(x, qkv_w, qkv_b, proj_w, proj_b, t_h=14, t_w=14, s_h=28, s_w=28, **kw):
    nc = _get_program()
    in_maps = _prep_inputs(x, qkv_w, qkv_b, proj_w, proj_b)
    res = bass_utils.run_bass_kernel_spmd(nc, in_maps, core_ids=list(range(N_CORES)))
    out = np.concatenate([res.results[c]["out"] for c in range(N_CORES)], axis=0)
    return out.astype(np.float32)


if __name__ == "__main__":
    build_program()
    print("program built OK")
